# revision 30
# baseline (speedup 1.0000x reference)
"""Multi-head attention (B=2, S=2048, D=768, H=12) on 8 TRN2 NeuronCores.

Sharding: core c -> batch b = c//4, head-group g = c%4 (3 heads of 64 each).

v3 design (vs v2.2 ~224-270us):
  - Q/K path in fp8e4 with DoubleRow matmuls (2 contraction planes per
    partition, 0.5 cyc/row): q/k projections contract 768 = 3 x (2x128)
    pairs; energy contracts 64 = 32x2 head-dim planes. Weights scaled x32
    so fp8e4 stays in normal range; exp scale absorbs the /1024.
  - q8/k8 layout [96, 2, SEQ]: head h at partitions 32h..32h+31, head-dim
    d = 32*plane + p. Removes the v2 qB2/kB2 duplicated-half hack.
  - V path / PV / out-projection stay bf16 (fp8 there would breach the
    2e-2 error budget).
  - Softmax: exp on ACT; a tunable subset of (gg, h) units computed on DVE
    as exact 2nd-order Taylor (t = s*e/sqrt2 + 1/sqrt2; P = (t*t + .5)*m)
    to balance ACT vs DVE.
  - reciprocal -> reciprocal_approx_fast (f32); broadcast matmuls via
    f32r bitcast (1 cyc/row).
  - PSUM->SBUF staging (ou, o_sb, vaug) on gpsimd; out-store DMA + weights
    on sync hwDGE; bulk x/mask on gpsimd swDGE with priority ordering.
"""

import os
import sys

sys.path.insert(0, "/opt/trn_rl_repo")

from contextlib import ExitStack

import ml_dtypes
import numpy as np

import concourse.bass as bass
import concourse.mybir as mybir
import concourse.tile as tile
from concourse import bacc
from concourse.bass import ds
from concourse.bass_utils import run_bass_kernel_spmd
from concourse.masks import make_identity

# ---- custom fused DVE op: P = ((e + B)^2 + C) * mask -----------------------
# Exact 2nd-order Taylor of exp(s*e) scaled by 1/a^2 (a = s/sqrt2); the
# common scale cancels in softmax normalization, so heads computed via this
# op are exact as long as the WHOLE head-column uses it.
from concourse import dve_ops as _dve_ops
from concourse.dve_spec import (
    Spec as _Spec, Src0 as _Src0, Src1 as _Src1, C0 as _C0, C1 as _C1,
    lower as _dve_lower, _has_src1 as _dve_has_src1, sq as _sq,
)
from concourse.dve_uop import DveOpSpec as _DveOpSpec


def _make_exp2m():
    name = "EXP2M_ANT"
    for op in _dve_ops.OPS:
        if op.name == name:
            return op
    spec = _Spec(
        body=(_sq(_Src0 + _C0) + _C1) * _Src1,
        reference=lambda in0, in1, s0, s1, imm2: (
            ((in0.astype(np.float32) + s0) ** 2 + s1) * in1
        ).astype(np.float32),
    )
    shas = {}
    for ver in ("v3", "v4"):
        d = _DveOpSpec(name=name, opcode=0, uops=_dve_lower(spec, ver=ver),
                       rd1_en=_dve_has_src1(spec))
        shas[ver] = d.sha(ver)
    op = _dve_ops.DveOp(name, spec, subdim=False, uops_sha=shas)
    _dve_ops.OPS.append(op)
    _dve_ops.CUSTOM_DVE_SPECS[name] = spec
    _dve_ops._SUB_OPCODE_FOR_NAME[name] = max(
        _dve_ops._SUB_OPCODE_FOR_NAME.values()) + 1
    assert _dve_ops._SUB_OPCODE_FOR_NAME[name] < 0x20
    return op


EXP2M = _make_exp2m()

F32 = mybir.dt.float32
F32R = mybir.dt.float32r
BF16 = mybir.dt.bfloat16
F8 = mybir.dt.float8e4

SEQ = 2048
D = 768
HD = 64
GD = 192
QB = 512
NQB = SEQ // QB   # 4
KT = SEQ // 128   # 16
NG = 8            # 2-ktile groups per block
W8SCALE = 32.0
SCALE = float(1.0 / np.sqrt(np.float32(D)))
S8 = SCALE / (W8SCALE * W8SCALE)   # exp scale for fp8 energies
EXP2M_B = float(1.0 / S8)          # (e + B)^2 + C form of the Taylor exp
EXP2M_C = float(1.0 / (S8 * S8))
PV_LAG = 2

# heads whose softmax runs on DVE via the fused EXP2M op (whole head-column
# must share the path so the 1/a^2 scale cancels in normalization)
FUSED_HEADS = {2}

_CACHE = {}


def _install_profile_hook():
    import types

    if "antenv.axon_hooks" in sys.modules:
        return
    sys.path.insert(0, "/root/.axon_site")
    try:
        from trn_agent_boot.trn_boot import _ntff_profile_via_ctypes
        hook = _ntff_profile_via_ctypes("/opt/axon/libaxon_pjrt.so")
    except Exception:
        hook = None
    import concourse.bass_utils as _bu

    _bu.upload_artifacts = lambda tmpdir: tmpdir
    mod = types.ModuleType("antenv.axon_hooks")
    mod.get_axon_ntff_profile_hook = lambda: hook
    mod.set_axon_ntff_profile_hook = lambda h: None
    sys.modules["antenv.axon_hooks"] = mod


def _build():
    nc = bacc.Bacc(None)
    Exp = mybir.ActivationFunctionType.Exp
    MUL = mybir.AluOpType.mult
    ADD = mybir.AluOpType.add
    DR = mybir.MatmulPerfMode.DoubleRow

    xq = nc.declare_dram_parameter("xq", [3, 128, 2, SEQ], F8, isOutput=False)
    xk = nc.declare_dram_parameter("xk", [3, 128, 2, SEQ], F8, isOutput=False)
    xv = nc.declare_dram_parameter("xv", [6, 128, SEQ], BF16, isOutput=False)
    wq = nc.declare_dram_parameter("wq", [3, 128, 2, GD], F8, isOutput=False)
    wk = nc.declare_dram_parameter("wk", [3, 128, 2, GD], F8, isOutput=False)
    wv = nc.declare_dram_parameter("wv", [128, 6, GD], BF16, isOutput=False)
    woT = nc.declare_dram_parameter("woT", [GD, D], BF16, isOutput=False)
    maskT = nc.declare_dram_parameter("maskT", [SEQ, SEQ], BF16, isOutput=False)
    out = nc.declare_dram_parameter("out", [SEQ, D], F32, isOutput=True)

    with tile.TileContext(nc) as tc, ExitStack() as ctx:
        cpool = ctx.enter_context(tc.tile_pool(name="const", bufs=1))
        ident = cpool.tile([128, 128], BF16)
        make_identity(nc, ident[:])

        # ---- persistent SBUF -----------------------------------------------
        pp = ctx.enter_context(tc.tile_pool(name="persist", bufs=1))
        # per-head [128, 2, SEQ]: plane 0 partitions 0..63 = head dims, rest
        # zeros (DoubleRow wants full 128-row tiles; zero-padded contraction
        # costs nothing — matmul time only tracks output columns)
        q8 = [pp.tile([128, 2, SEQ], F8, tag=f"q8{h}", name=f"q8_{h}")
              for h in range(3)]
        k8 = [pp.tile([128, 2, SEQ], F8, tag=f"k8{h}", name=f"k8_{h}")
              for h in range(3)]
        vaug = [pp.tile([128, KT, HD + 1], BF16, tag=f"vaug{h}", name=f"vaug{h}")
                for h in range(3)]
        onormA = pp.tile([128, SEQ], BF16, tag="onA")
        onormB = pp.tile([64, SEQ], BF16, tag="onB")
        woA_sb = pp.tile([128, D], BF16, tag="woA")
        woB_sb = pp.tile([64, D], BF16, tag="woB")
        ones65b = pp.tile([65, HD], BF16, tag="ones65b")

        # fp8 proj weights: [128, 2, GD] per k-pair j
        w8_sb = {}
        for name, wT in (("q", wq), ("k", wk)):
            w8_sb[name] = [
                pp.tile([128, 2, GD], F8, tag=f"w8{name}{j}", name=f"w8_{name}{j}")
                for j in range(3)
            ]
        wv_sb = pp.tile([128, 6, GD], BF16, tag="wv")

        # weight DMAs on sync hwDGE (small, early)
        for name, wT in (("q", wq), ("k", wk)):
            for j in range(3):
                nc.sync.dma_start(w8_sb[name][j][:], wT[j, :, :, :])
        nc.sync.dma_start(wv_sb[:], wv[:, :, :])
        nc.sync.dma_start(woA_sb[:], woT[0:128, :])
        nc.sync.dma_start(woB_sb[:], woT[128:GD, :])

        for h in range(3):
            nc.vector.memset(vaug[h][:, :, HD:HD + 1], 1.0)
            # zero the unused DoubleRow planes/rows (0 * garbage could be NaN)
            nc.gpsimd.memset(q8[h][:, 1, :], 0.0)
            nc.gpsimd.memset(k8[h][:, 1, :], 0.0)
            nc.gpsimd.memset(q8[h][64:128, 0, :], 0.0)
            nc.gpsimd.memset(k8[h][64:128, 0, :], 0.0)
        nc.vector.memset(ones65b[:], 1.0)

        mp = ctx.enter_context(tc.tile_pool(name="mp", bufs=2))
        maskR = maskT.rearrange("(j p) q -> p j q", p=128)
        mask_t = {}

        def issue_mask(n):
            """16 single-ktile chunks -> full 16-queue spread."""
            mask_t[n] = mp.tile([128, KT, QB], BF16, tag="mask", name=f"mask{n}")
            for j in range(KT):
                nc.gpsimd.dma_start(
                    mask_t[n][:, ds(j, 1), :], maskR[:, ds(j, 1), ds(n * QB, QB)]
                )

        dummy = cpool.tile([1, 2], F32)
        nc.scalar.activation(dummy[:], ident[0:1, 0:2], Exp, scale=1.0)

        # ---- phase A: projections ------------------------------------------
        with tc.tile_pool(name="xs8", bufs=1) as xs8, \
             tc.tile_pool(name="xsv", bufs=1) as xsv, \
             tc.tile_pool(name="vtp", bufs=1) as vtp:
            x8_sb = {}
            for name in ("q", "k"):
                x8_sb[name] = [
                    xs8.tile([128, 2, SEQ], F8, tag=f"x8{name}{j}",
                             name=f"x8_{name}{j}")
                    for j in range(3)
                ]
            xv_sb = [
                xsv.tile([128, SEQ], BF16, tag=f"xv{k}", name=f"x_v{k}")
                for k in range(6)
            ]
            # DMA priority order: q j0, k j0 fine chunks first; then mask0;
            # then q/k j1-2; then v; mask1 issued at end of phase A.
            xsrc8 = {"q": xq, "k": xk}
            for name in ("q", "k"):
                for c in range(8):
                    nc.gpsimd.dma_start(
                        x8_sb[name][0][:, :, ds(c * 256, 256)],
                        xsrc8[name][ds(0, 1), :, :, ds(c * 256, 256)],
                    )
            issue_mask(0)
            for name in ("q", "k"):
                for j in (1, 2):
                    for c in range(4):
                        nc.gpsimd.dma_start(
                            x8_sb[name][j][:, :, ds(c * 512, 512)],
                            xsrc8[name][ds(j, 1), :, :, ds(c * 512, 512)],
                        )
            for k in range(6):
                for c in range(4):
                    nc.gpsimd.dma_start(
                        xv_sb[k][:, ds(c * 512, 512)],
                        xv[ds(k, 1), :, ds(c * 512, 512)],
                    )

            def proj8(name, dst, pj8):
                """fp8 DoubleRow projection into dst[h] [64, 2, SEQ] plane 0.

                Per (n, h): psum [64, 512] at base partition 0; mm per j:
                lhsT = w8[j][:, :, 64h:64h+64] (free 2x64), rhs =
                x8[j][:, :, n slice] (free 2x512).
                """
                for n in range(NQB):
                    for h in range(3):
                        ps = pj8.tile([64, QB], F32, tag=f"p8{h}",
                                      name=f"p8_{name}{n}_{h}")
                        for j in range(3):
                            nc.tensor.matmul(
                                ps[:],
                                lhsT=w8_sb[name][j][:, :, ds(64 * h, 64)],
                                rhs=x8_sb[name][j][:, :, ds(n * QB, QB)],
                                start=(j == 0), stop=(j == 2),
                                perf_mode=DR,
                            )
                        if h < 2:
                            nc.vector.tensor_copy(
                                dst[h][0:64, 0, ds(n * QB, QB)], ps[:])
                        else:
                            nc.scalar.copy(
                                dst[h][0:64, 0, ds(n * QB, QB)], ps[:])

            def projv(pjA, pjB, tr_ps):
                """bf16 v projection (as v2): vtA [128, SEQ], vtB [64, SEQ]."""
                vtA = vtp.tile([128, SEQ], BF16, tag="vtA")
                vB2 = vtp.tile([128, SEQ], BF16, tag="vB2")
                vtB = vtp.tile([64, SEQ], BF16, tag="vtB")
                psA = [pjA.tile([128, QB], F32, tag=f"pA{n}", name=f"pA_v{n}")
                       for n in range(NQB)]
                psB = [pjB.tile([128, QB], F32, tag=f"pB{p}", name=f"pB_v{p}")
                       for p in range(NQB // 2)]
                for k in range(6):
                    for n in range(NQB):
                        nc.tensor.matmul(
                            psA[n][:],
                            lhsT=wv_sb[:, k, 0:128],
                            rhs=xv_sb[k][:, ds(n * QB, QB)],
                            start=(k == 0), stop=(k == 5),
                        )
                    for p in range(NQB // 2):
                        for half in range(2):
                            n = 2 * p + half
                            nc.tensor.matmul(
                                psB[p][ds(half * 64, 64), :],
                                lhsT=wv_sb[:, k, 128:GD],
                                rhs=xv_sb[k][:, ds(n * QB, QB)],
                                start=(k == 0), stop=(k == 5),
                                tile_position=(0, half * 64),
                            )
                for n in range(NQB):
                    nc.scalar.copy(vtA[:, ds(n * QB, QB)], psA[n][:])
                for p in range(NQB // 2):
                    for half in range(2):
                        n = 2 * p + half
                        b0 = half * 64
                        nc.scalar.copy(
                            vB2[b0:b0 + 64, ds(n * QB, QB)],
                            psB[p][b0:b0 + 64, :])
                for n in range(NQB):
                    src = (n % 2) * 64
                    nc.gpsimd.dma_start(
                        vtB[:, ds(n * QB, QB)], vB2[src:src + 64, ds(n * QB, QB)])
                for s in range(KT):
                    ptA = tr_ps.tile([128, 128], BF16, tag="ptA")
                    nc.tensor.transpose(ptA[:], vtA[:, ds(s * 128, 128)], ident[:])
                    nc.vector.tensor_copy(vaug[0][:, s, 0:HD], ptA[:, 0:64])
                    nc.vector.tensor_copy(vaug[1][:, s, 0:HD], ptA[:, 64:128])
                    ptB = tr_ps.tile([128, 64], BF16, tag="ptB")
                    nc.tensor.transpose(
                        ptB[:], vtB[0:64, ds(s * 128, 128)], ident[0:64, 0:64])
                    nc.vector.tensor_copy(vaug[2][:, s, 0:HD], ptB[:, 0:64])

            with tc.tile_pool(name="pj8", bufs=2, space="PSUM") as pj8:
                proj8("q", q8, pj8)
                proj8("k", k8, pj8)
            with tc.tile_pool(name="pjA", bufs=1, space="PSUM") as pjA, \
                 tc.tile_pool(name="pjB", bufs=1, space="PSUM") as pjB, \
                 tc.tile_pool(name="tr_ps", bufs=1, space="PSUM") as tr_ps:
                projv(pjA, pjB, tr_ps)
            issue_mask(1)

        # ---- phase B: fused attention + out-projection ---------------------
        pp2 = ctx.enter_context(tc.tile_pool(name="pp2", bufs=2))
        rp = ctx.enter_context(tc.tile_pool(name="rp", bufs=2))
        op = ctx.enter_context(tc.tile_pool(name="op", bufs=3))
        sg = ctx.enter_context(tc.tile_pool(name="sg", bufs=1))

        with tc.tile_pool(name="e_ps", bufs=2, space="PSUM") as e_ps, \
             tc.tile_pool(name="ou_ps", bufs=1, space="PSUM") as ou_ps, \
             tc.tile_pool(name="f_ps", bufs=1, space="PSUM") as f_ps:

            P = {}
            ou = {}
            ou_sb = {}

            def e_mms(n, g, h):
                """fp8 DoubleRow energy: e[128 kpos, 2, 512]."""
                e = e_ps.tile([128, 2, QB], F32, tag="e", name=f"e{n}_{g}_{h}")
                for mm in range(2):
                    m = 2 * g + mm
                    nc.tensor.matmul(
                        e[:, mm, :],
                        lhsT=k8[h][:, :, ds(m * 128, 128)],
                        rhs=q8[h][:, :, ds(n * QB, QB)],
                        start=True, stop=True,
                        perf_mode=DR,
                    )
                return e

            def exp_op(n, g, h, e):
                if h in FUSED_HEADS:
                    # one DVE op: P = ((e + B)^2 + C) * mask  (exp+mask fused)
                    nc.vector._custom_dve(
                        EXP2M,
                        out=P[n, h][:, ds(2 * g, 2), :],
                        in0=e[:, :, :],
                        in1=mask_t[n][:, ds(2 * g, 2), :],
                        s0=EXP2M_B, s1=EXP2M_C,
                    )
                else:
                    nc.scalar.activation(
                        P[n, h][:, ds(2 * g, 2), :], e[:, :, :], Exp, scale=S8)

            def mul_op(n, gg, h):
                sl = ds(4 * gg, 4)
                nc.vector.tensor_mul(
                    P[n, h][:, sl, :], P[n, h][:, sl, :], mask_t[n][:, sl, :])

            def pv_mms(n, g, h):
                for mm in range(2):
                    m = 2 * g + mm
                    nc.tensor.matmul(
                        ou[n, h][:],
                        lhsT=vaug[h][:, m, :],
                        rhs=P[n, h][:, m, :],
                        start=(m == 0), stop=(m == KT - 1),
                    )

            def stage_head(n, h):
                """stage ou to SBUF (bf16) and park denominator row at psum
                partition 32h (bf16 matmul — v2.2-proven geometry)."""
                if h == 0:
                    r3cur[n] = f_ps.tile([65, QB], F32, tag="f", name=f"r3_{n}")
                ou_sb[n, h] = sg.tile(
                    [HD + 1, QB], BF16, tag=f"os{h}", name=f"os{n}_{h}")
                nc.scalar.copy(ou_sb[n, h][:], ou[n, h][:])
                nc.tensor.matmul(
                    r3cur[n][32 * h:32 * h + 1, :],
                    lhsT=ones65b[64:65, 0:1],
                    rhs=ou_sb[n, h][HD:HD + 1, :],
                    start=True, stop=True)

            def stage_fin(n):
                from concourse.dve_ops import (
                    RECIP_APPROX_FAST_CONSTS as _RC, RECIPROCAL_APPROX_FAST)
                ri = rp.tile([65, QB], BF16, tag="ri", name=f"ri_{n}")
                nc.vector._custom_dve(
                    RECIPROCAL_APPROX_FAST, out=ri[:], in0=r3cur[n][:],
                    s0=_RC["s0"], s1=_RC["s1"], imm2=_RC["imm2"])
                return ri

            r3cur = {}

            def norm_op(n, h, ri):
                nsl = ds(n * QB, QB)
                rbps = f_ps.tile([64, QB], F32, tag="f", name=f"rb_{n}_{h}")
                nc.tensor.matmul(
                    rbps[:],
                    lhsT=ones65b[32 * h:32 * h + 1, 0:64],
                    rhs=ri[32 * h:32 * h + 1, :],
                    start=True, stop=True)
                if h == 0:
                    nc.vector.tensor_mul(
                        onormA[0:64, nsl], ou_sb[n, h][0:HD, :], rbps[:])
                elif h == 1:
                    tmp1 = rp.tile([64, QB], BF16, tag="t1", name=f"t1_{n}")
                    nc.vector.tensor_mul(tmp1[:], ou_sb[n, h][0:HD, :], rbps[:])
                    nc.gpsimd.dma_start(onormA[64:128, nsl], tmp1[:])
                else:
                    nc.vector.tensor_mul(
                        onormB[:, nsl], ou_sb[n, h][0:HD, :], rbps[:])

            def outproj(n, mqi, split_dma=False):
                mq = 4 * n + mqi
                msl = ds(mq * 128, 128)
                o_sb = op.tile([128, D], F32, tag="o", name=f"o{mq}")
                for n0, nw in ((0, 512), (512, 256)):
                    if split_dma and n0 == 512:
                        fpt = e_ps.tile(
                            [128, 2, QB], F32, tag="e", name=f"fp{mq}_{n0}"
                        )[:].rearrange("p a b -> p (a b)")[:, 0:QB]
                    else:
                        fpt = f_ps.tile(
                            [128, QB], F32, tag="f", name=f"fp{mq}_{n0}")
                    nc.tensor.matmul(
                        fpt[:, 0:nw], lhsT=onormA[:, msl],
                        rhs=woA_sb[:, ds(n0, nw)], start=True, stop=False)
                    nc.tensor.matmul(
                        fpt[:, 0:nw], lhsT=onormB[:, msl],
                        rhs=woB_sb[:, ds(n0, nw)], start=False, stop=True)
                    nc.scalar.copy(o_sb[:, ds(n0, nw)], fpt[:, 0:nw])
                if split_dma:
                    for c0 in (0, 256, 512):
                        nc.sync.dma_start(
                            out[msl, ds(c0, 256)], o_sb[:, ds(c0, 256)])
                else:
                    nc.sync.dma_start(out[msl, :], o_sb[:])

            ri_prev = None
            for n in range(NQB):
                if 1 <= n < NQB - 1:
                    issue_mask(n + 1)
                for h in range(3):
                    P[n, h] = pp2.tile(
                        [128, KT, QB], BF16, tag=f"P{h}", name=f"P{n}_{h}")
                    ou[n, h] = ou_ps.tile(
                        [HD + 1, QB], F32, tag=f"ou{h}", name=f"ou{n}_{h}")

                for g in range(NG):
                    for h in range(3):
                        e = e_mms(n, g, h)
                        exp_op(n, g, h, e)
                    if g % 2 == 1:
                        for h in range(3):
                            if h not in FUSED_HEADS:
                                mul_op(n, g // 2, h)
                    gl = g - PV_LAG
                    if gl >= 0:
                        for h in range(3):
                            pv_mms(n, gl, h)
                    # previous block's normalization + out-projection
                    if n >= 1:
                        if g == 2:
                            for h in range(3):
                                norm_op(n - 1, h, ri_prev)
                        elif 3 <= g < 7:
                            outproj(n - 1, g - 3)
                for gl in range(NG - PV_LAG, NG):
                    for h in range(3):
                        pv_mms(n, gl, h)
                for h in range(3):
                    stage_head(n, h)
                ri_prev = stage_fin(n)

            for h in range(3):
                norm_op(NQB - 1, h, ri_prev)
            for mqi in range(4):
                outproj(NQB - 1, mqi, split_dma=True)

    nc.compile()
    return nc


def kernel(Q, K, V, mask, Wq, Wk, Wv, Wo):
    if "nc" not in _CACHE:
        _CACHE["nc"] = _build()
    nc = _CACHE["nc"]

    FP8 = ml_dtypes.float8_e4m3
    maskT_bf = np.ascontiguousarray(
        (mask[0, 0].T != 0).astype(ml_dtypes.bfloat16)
    )
    in_maps = []
    for c in range(8):
        b, g = c // 4, c % 4
        sl = slice(g * GD, (g + 1) * GD)

        def prep_x8(X):
            # [3, 128, 2, SEQ]: plane i = k-tile 2j+i
            xt = X[b].T.reshape(3, 2, 128, SEQ).transpose(0, 2, 1, 3)
            return np.ascontiguousarray(xt.astype(FP8))

        def prep_w8(W):
            # [3, 128, 2, GD]
            wT = (W[sl, :].T * W8SCALE).reshape(3, 2, 128, GD)
            return np.ascontiguousarray(
                wT.transpose(0, 2, 1, 3).astype(FP8))

        def prep_xv(X):
            return np.ascontiguousarray(
                X[b].T.reshape(6, 128, SEQ).astype(ml_dtypes.bfloat16))

        def prep_wv(W):
            wT = W[sl, :].T.reshape(6, 128, GD)
            return np.ascontiguousarray(
                wT.transpose(1, 0, 2).astype(ml_dtypes.bfloat16))

        in_maps.append(
            {
                "xq": prep_x8(Q), "xk": prep_x8(K), "xv": prep_xv(V),
                "wq": prep_w8(Wq), "wk": prep_w8(Wk), "wv": prep_wv(Wv),
                "woT": np.ascontiguousarray(
                    Wo[:, sl].T.astype(ml_dtypes.bfloat16)),
                "maskT": maskT_bf,
            }
        )

    _install_profile_hook()
    res = run_bass_kernel_spmd(
        nc,
        in_maps,
        core_ids=list(range(8)),
        trace=bool(int(os.environ.get("KERNEL_PROFILE", "0"))),
    )
    _CACHE["last_exec_ns"] = res.exec_time_ns
    _CACHE["last_res"] = res

    outp = np.zeros((2, SEQ, D), dtype=np.float32)
    for c in range(8):
        outp[c // 4] += res.results[c]["out"]
    return outp
